# revision 60
# baseline (speedup 1.0000x reference)
"""Trainium2 Bass kernel for nn_ContrastiveLearning (self-contained).

kernel(**inputs) takes the FULL unsharded inputs (as produced by the
problem's setup_inputs) and returns (logits_per_img, logits_per_depth),
each [4, 100, 100] fp32.

Sharding: 8 NeuronCores, core c = (batch b=c//2, modality m=c%2). Each core
streams its 26 MB feature map in 40 half-patch-row DMAs, computes the
conv1x1 directly in transposed [pixel, patch] layout via per-patch
matmuls (patch slab as stationary, conv weight as 1-column moving),
runs the patch MLP + LayerNorm per patch-row chunk pipelined under the
DMA stream, exchanges eT with its pair partner via a 2-core AllGather,
and computes the 100x100 contrastive logits on-device. exp(logit_scale)
is folded into the img branch's LayerNorm affine on the host.
"""
import numpy as np
import concourse.bass as bass
import concourse.bacc as bacc
import concourse.mybir as mybir
import concourse.tile as tile
from concourse.bass_utils import run_bass_kernel_spmd


F32 = mybir.dt.float32
F32R = mybir.dt.float32r
AF = mybir.ActivationFunctionType
ALU = mybir.AluOpType

NV = NH = 10          # patch grid
NP = NV * NH          # 100 patches
CPS = 16
ENC = 128
PIX = CPS * CPS       # 256 features per patch
LN_EPS = 1e-5
HH = CPS // 2         # h-half rows per DMA unit

# consts column layout:
#   [bias | w1t(512) | w2t(256) | w2colmean(2) | g_row | (-g)_row | b_row]
#   (the three *_row blocks live on partition 0 only)
C_BIAS = 0
C_W1 = 1
C_W2 = C_W1 + 512
C_CM = C_W2 + 256
C_GR = C_CM + 2
C_GN = C_GR + 128
C_BR = C_GN + 128
C_TOT = C_BR + 128


def build_kernel(nc, n_cores=8, exchange='cc', debug=False):
    feat = nc.dram_tensor("feat", [2, 128, 160, 160], F32, kind="ExternalInput")
    convw = nc.dram_tensor("convw", [128, 128], F32, kind="ExternalInput")
    consts = nc.dram_tensor("consts", [128, C_TOT], F32, kind="ExternalInput")
    logits = nc.dram_tensor("logits", [NP, NP], F32, kind="ExternalOutput")
    dbg = None
    if debug:
        dbg = nc.dram_tensor("dbg", [128, 3 * NP + 50], F32, kind="ExternalOutput")

    NPA = 9 * NH          # patches exchanged in the early round
    NPB = NP - NPA
    cc_in1 = cc_out1 = cc_in2 = cc_out2 = None
    if exchange in ('cc', 'local'):
        cc_in1 = nc.dram_tensor("cc_in1", [ENC, NPA], F32)
        cc_out1 = nc.dram_tensor("cc_out1", [2 * ENC, NPA], F32)
        cc_in2 = nc.dram_tensor("cc_in2", [ENC, NPB], F32)
        cc_out2 = nc.dram_tensor("cc_out2", [2 * ENC, NPB], F32)

    with tile.TileContext(nc) as tc:
        with (
            tc.tile_pool(name="cst", bufs=1) as cst,
            tc.tile_pool(name="slab", bufs=3) as slab_pool,
            tc.tile_pool(name="slab2", bufs=3) as slab2_pool,
            tc.tile_pool(name="chunk", bufs=2) as chk,
            tc.tile_pool(name="work", bufs=1) as work,
            tc.tile_pool(name="pxp", bufs=2, space="PSUM") as pxp,
            tc.tile_pool(name="php", bufs=1, space="PSUM") as php,
            tc.tile_pool(name="pyp", bufs=1, space="PSUM") as pyp,
            tc.tile_pool(name="pst", bufs=2, space="PSUM") as pstp,
            tc.tile_pool(name="pbc", bufs=1, space="PSUM") as pbc,
            tc.tile_pool(name="plg", bufs=1, space="PSUM") as plg,
        ):
            # constants first so their DMAs clear before the feature stream
            convw_s = cst.tile([128, 128], F32, tag="convw")
            cst_s = cst.tile([128, C_TOT], F32, tag="consts")
            nc.scalar.dma_start(convw_s[:], convw[:])
            nc.scalar.dma_start(cst_s[:], consts[:])
            bias_s = cst_s[:, C_BIAS:C_BIAS + 1]
            g_row = cst_s[0:1, C_GR:C_GR + 128]
            gn_row = cst_s[0:1, C_GN:C_GN + 128]
            b_row = cst_s[0:1, C_BR:C_BR + 128]

            oc = cst.tile([128, 1], F32, tag="oc")
            nc.vector.memset(oc[:], 1.0 / ENC)
            eps_s = cst.tile([1, 1], F32, tag="eps")
            nc.vector.memset(eps_s[:], LN_EPS)
            ones_nh = cst.tile([1, NH], F32, tag="ones_nh")
            nc.vector.memset(ones_nh[:], 1.0)

            eT = work.tile([128, NP], F32, tag="eT")

            def w1blk(u, v):
                o = C_W1 + 256 * u + 128 * v
                return cst_s[:, o:o + 128]

            def w2blk(u):
                o = C_W2 + 128 * u
                return cst_s[:, o:o + 128]

            # software-pipelined: phase A(r) = conv+MLP on PE; phase B(r) =
            # LN stats/broadcast matmuls, issued after A(r+1) so PE never
            # head-of-line blocks on the LN vector chain.
            slabs = []
            slabs2 = []
            chunks = []

            def dma_row(r):
                st = slab_pool.tile([128, 2, 2, HH, 160], F32, tag="st")
                # the very last half-row lands in two quarters so its
                # rearrange can overlap the end of the stream
                nq = 2 if r == NV - 1 else 1
                for hh in range(2):
                    qs = nq if hh == 1 else 1
                    qh = HH // qs
                    for q in range(qs):
                        row0 = r * CPS + hh * HH + q * qh
                        nc.sync.dma_start(
                            st[:, hh, :, q * qh:(q + 1) * qh, :],
                            feat[:, :, row0:row0 + qh, :].rearrange(
                                "u c h w -> c u h w"),
                        )
                slabs.append(st)

            def rearrange_row(r):
                # patch-major copy so the per-patch conv stationary collapses
                # to a single contiguous free dim (walrus LDWEIGHTS rule)
                st = slabs[r]
                st2 = slab2_pool.tile([128, 2, 2, NH, HH, CPS], F32, tag="st2")
                nq = 2 if r == NV - 1 else 1
                # engine split keeps every copy off the tail-critical queues:
                # DVE takes u0, Pool takes hh0/u1, Act takes hh1/u1
                for hh in range(2):
                    qs = nq if hh == 1 else 1
                    qh = HH // qs
                    for q in range(qs):
                        for u in range(2):
                            src = st[:, hh, u, q * qh:(q + 1) * qh, :].rearrange(
                                "c h (p w) -> c p h w", w=CPS)
                            dst = st2[:, hh, u, :, q * qh:(q + 1) * qh, :]
                            if u == 0:
                                nc.vector.tensor_copy(dst, src)
                            elif hh == 0:
                                nc.gpsimd.tensor_copy(dst, src)
                            else:
                                nc.scalar.activation(dst, src, AF.Copy)
                slabs2.append(st2)

            def phase_a(r):
                st2 = slabs2[r]
                px = pxp.tile([128, 2, NH], F32, tag="px")
                xc = chk.tile([128, 2, NH], F32, tag="xc")
                ph = php.tile([128, 2, NH], F32, tag="ph")
                # i-half hh's conv runs as soon as its h-half DMA lands; the
                # previous i-half's evac + MLP1 matmuls are issued first so
                # they fill PE time while the next half-row DMA is in flight.
                for hh in range(2):
                    for c in range(NH):
                        for u in range(2):
                            nc.tensor.matmul(
                                px[:, hh, c:c + 1],
                                st2[:, hh, u, c, :, :],
                                convw_s[:, u:u + 1],
                                start=(u == 0), stop=(u == 1),
                            )
                    nc.scalar.activation(xc[:, hh, :], px[:, hh, :], AF.Relu,
                                         bias=bias_s)
                # accumulation groups must not interleave on PE: keep each
                # start/stop pair back-to-back
                for v in range(2):
                    for u in range(2):
                        nc.tensor.matmul(ph[:, v, :], w1blk(u, v), xc[:, u, :],
                                         start=(u == 0), stop=(u == 1))
                hc = chk.tile([128, 2, NH], F32, tag="hc")
                nc.scalar.activation(hc[:, :, :], ph[:, :, :], AF.Relu)
                # mean of y comes from hc via w2 column-means (off critical path)
                ps = pstp.tile([1, 2, NH], F32, tag="ps")
                py = pyp.tile([128, NH], F32, tag="py")
                for u in range(2):
                    nc.tensor.matmul(ps[0:1, 0, :], cst_s[:, C_CM + u:C_CM + u + 1],
                                     hc[:, u, :], start=(u == 0), stop=(u == 1))
                for u in range(2):
                    nc.tensor.matmul(py[:], w2blk(u), hc[:, u, :],
                                     start=(u == 0), stop=(u == 1))
                yc = chk.tile([128, NH], F32, tag="yc")
                nc.vector.tensor_copy(yc[:], py[:])
                sq = chk.tile([128, NH], F32, tag="sq")
                nc.scalar.activation(sq[:], py[:], AF.Square)
                if debug and r == 0:
                    o = 3 * NP
                    nc.scalar.dma_start(dbg[:, o:o + 20], xc[:, :, :])
                    nc.scalar.dma_start(dbg[:, o + 20:o + 40], hc[:, :, :])
                    nc.scalar.dma_start(dbg[:, o + 40:o + 50], yc[:])
                chunks.append((yc, sq, ps))

            def phase_b(r):
                yc, sq, ps = chunks[r]
                cols = slice(r * NH, (r + 1) * NH)
                nc.tensor.matmul(ps[0:1, 1, :], oc[:], sq[:], start=True, stop=True)
                m2 = chk.tile([1, NH], F32, tag="m2")
                nc.scalar.activation(m2[:], ps[0:1, 0, :], AF.Square)
                veps = chk.tile([1, NH], F32, tag="veps")
                nc.vector.tensor_tensor(veps[:], ps[0:1, 1, :], m2[:], ALU.subtract)
                srow = chk.tile([1, NH], F32, tag="srow")
                nc.scalar.activation(srow[:], veps[:], AF.Sqrt, bias=eps_s[:])
                rrow = chk.tile([1, NH], F32, tag="rrow")
                nc.vector.reciprocal(rrow[:], srow[:])
                tmr = chk.tile([1, NH], F32, tag="tmr")
                nc.vector.tensor_tensor(tmr[:], ps[0:1, 0, :], rrow[:], ALU.mult)
                pb = pbc.tile([128, 2, NH], F32, tag="pb")
                nc.tensor.matmul(pb[:, 0, :], g_row, rrow[:], start=True, stop=True)
                nc.tensor.matmul(pb[:, 1, :], gn_row, tmr[:], start=True, stop=False)
                nc.tensor.matmul(pb[:, 1, :], b_row, ones_nh[:], start=False,
                                 stop=True)
                t1 = chk.tile([128, NH], F32, tag="t1")
                nc.vector.tensor_tensor(t1[:], yc[:], pb[:, 0, :], ALU.mult)
                nc.vector.tensor_tensor(eT[:, cols], t1[:], pb[:, 1, :], ALU.add)

            groups = [[2 * i, 2 * i + 1] for i in range(n_cores // 2)]
            B2 = work.tile([128, 2, NP], F32, tag="B2")
            pL = plg.tile([NP, NP], F32, tag="pL")
            L_s = work.tile([NP, NP], F32, tag="Ls")
            if exchange == 'rdma':
                rsem = nc.alloc_semaphore("rdma_rsem")
                lsem = nc.alloc_semaphore("rdma_lsem")

            def Aop(lo, hi):
                # stationary side of the logits matmuls: local eT when the
                # partner swap is one-way (rdma), gathered img half otherwise
                if exchange in ('rdma', 'rlocal', 'none'):
                    return eT[:, lo:hi]
                return B2[:, 0, lo:hi]

            def exchange_round(lo, hi, ci, co):
                # queue choice keeps every hop off the queues running the LN
                # chain: round-1 write on gpsimd (idle), round-2 write on
                # vector (dispatches right after the final eT write there),
                # gather on gpsimd, readbacks on sync.
                wq = nc.gpsimd if lo == 0 else nc.scalar
                if exchange == 'cc':
                    wq.dma_start(ci[:], eT[:, lo:hi])
                    nc.gpsimd.collective_compute(
                        "AllGather", ALU.bypass, replica_groups=groups,
                        ins=[ci.ap().opt()], outs=[co.ap().opt()],
                    )
                    nc.sync.dma_start(
                        B2[:, :, lo:hi],
                        co.ap().rearrange("(two p) n -> p two n", two=2))
                elif exchange == 'local':
                    # collective-free stand-in, same DRAM round-trip cost
                    wq.dma_start(ci[:], eT[:, lo:hi])
                    nc.gpsimd.dma_start(co[0:ENC, :], ci[:])
                    nc.gpsimd.dma_start(co[ENC:2 * ENC, :], ci[:])
                    nc.sync.dma_start(
                        B2[:, :, lo:hi],
                        co.ap().rearrange("(two p) n -> p two n", two=2))
                elif exchange == 'rdma':
                    # one-way swap: partner's eT lands in B2[:, 1, lo:hi];
                    # odd cores' logits are garbage and ignored by the host
                    nc.gpsimd.remote_dma_broadcast(
                        B2[:, 1, lo:hi], eT[:, lo:hi], remote_sem=rsem,
                        local_sem=lsem, rdests=[(0, 1)] + [None] * 7)
                    nc.gpsimd.trigger_dma(count=None)
                    with tc.tile_critical():
                        nc.gpsimd.wait_ge(rsem, 2 * (1 if hi == NPA else 2))
                        nc.gpsimd.tensor_copy(B2[:, 1, lo:hi], B2[:, 1, lo:hi])
                elif exchange == 'rlocal':
                    # sim stand-in for rdma: one SBUF->SBUF DMA hop
                    nc.sync.dma_start(B2[:, 1, lo:hi], eT[:, lo:hi])
                else:
                    nc.vector.tensor_copy(B2[:, 1, lo:hi], eT[:, lo:hi])

            dma_row(0)
            dma_row(1)
            rearrange_row(0)
            phase_a(0)
            for r in range(1, NV):
                if r + 1 < NV:
                    dma_row(r + 1)
                phase_b(r - 1)
                rearrange_row(r)
                phase_a(r)
                if r == NV - 1:
                    # early exchange of the first 9 patch-rows, hidden
                    # under the tail of the feature stream
                    exchange_round(0, NPA, cc_in1, cc_out1)
                    nc.tensor.matmul(pL[0:64, 0:NPA], Aop(0, 64),
                                     B2[:, 1, 0:NPA], start=True, stop=True)
                    nc.vector.tensor_copy(L_s[0:64, 0:NPA], pL[0:64, 0:NPA])
                    nc.scalar.dma_start(logits[0:64, 0:NPA], L_s[0:64, 0:NPA])
            phase_b(NV - 1)
            exchange_round(NPA, NP, cc_in2, cc_out2)

            # logits_img = (s*e1).T @ e2  (exp(logit_scale) folded into img g/b)
            # remaining blocks (out rows must start at a 0/32/64 boundary)
            nc.tensor.matmul(pL[0:64, NPA:NP], Aop(0, 64),
                             B2[:, 1, NPA:NP], start=True, stop=True)
            nc.tensor.matmul(pL[64:NP, 0:NPA], Aop(64, NP),
                             B2[:, 1, 0:NPA], start=True, stop=True)
            nc.tensor.matmul(pL[64:NP, NPA:NP], Aop(64, NP),
                             B2[:, 1, NPA:NP], start=True, stop=True)
            nc.scalar.activation(L_s[0:64, NPA:NP], pL[0:64, NPA:NP], AF.Copy)
            nc.vector.tensor_copy(L_s[64:NP, :], pL[64:NP, :])
            nc.scalar.dma_start(logits[0:64, NPA:NP], L_s[0:64, NPA:NP])
            nc.sync.dma_start(logits[64:NP, :], L_s[64:NP, :])
            if debug:
                nc.scalar.dma_start(dbg[:, 0:NP], eT[:])
                nc.scalar.dma_start(dbg[:, NP:2 * NP], B2[:, 0, :])
                nc.scalar.dma_start(dbg[:, 2 * NP:3 * NP], B2[:, 1, :])

    nc.compile()
    return nc


def host_inputs_for_core(core, inputs):
    """Build the per-core in_map from the full problem inputs dict."""
    b, m = core // 2, core % 2
    feat = np.asarray(inputs["feat_c1" if m == 0 else "feat_c2"])[b]
    pre = "img_" if m == 0 else "depth_"
    cw = np.zeros((128, 128), np.float32)
    cw[:, 0:2] = np.asarray(inputs[pre + "conv_w"]).reshape(2, 128).T
    w1 = np.asarray(inputs[pre + "w1"])  # [256,256] (o=128v+m', i=128u+k)
    w1t = np.ascontiguousarray(
        w1.reshape(2, 128, 2, 128).transpose(3, 2, 0, 1).reshape(128, 512))
    w2 = np.asarray(inputs[pre + "w2"])  # [128,256]
    w2t = np.ascontiguousarray(
        w2.reshape(128, 2, 128).transpose(2, 1, 0).reshape(128, 256))
    s = float(np.exp(np.asarray(inputs["logit_scale"]))) if m == 0 else 1.0
    g = np.asarray(inputs[pre + "ln_g"]) * s
    bb = np.asarray(inputs[pre + "ln_b"]) * s
    cst = np.zeros((128, C_TOT), np.float32)
    cst[:, C_BIAS] = np.asarray(inputs[pre + "conv_b"])[0]
    cst[:, C_W1:C_W1 + 512] = w1t
    cst[:, C_W2:C_W2 + 256] = w2t
    cst[:, C_CM:C_CM + 2] = (w2.sum(0).reshape(2, 128) / ENC).T
    cst[0, C_GR:C_GR + 128] = g
    cst[0, C_GN:C_GN + 128] = -g
    cst[0, C_BR:C_BR + 128] = bb
    return {
        "feat": np.ascontiguousarray(feat).reshape(2, 128, 160, 160),
        "convw": cw,
        "consts": cst,
    }


_NC_CACHE = {}


def _get_nc():
    if "nc" not in _NC_CACHE:
        import os
        exch = os.environ.get("KERNEL_EXCHANGE", "cc")
        nc = bacc.Bacc("TRN2", target_bir_lowering=False, num_devices=8)
        build_kernel(nc, n_cores=8, exchange=exch)
        _NC_CACHE["nc"] = nc
    return _NC_CACHE["nc"]


def kernel(**inputs):
    nc = _get_nc()
    in_maps = [host_inputs_for_core(c, inputs) for c in range(8)]
    res = run_bass_kernel_spmd(nc, in_maps, list(range(8)))
    logits_img = np.stack([np.asarray(res.results[2 * b]["logits"])
                           for b in range(4)]).astype(np.float32)
    logits_depth = np.ascontiguousarray(logits_img.transpose(0, 2, 1))
    return logits_img, logits_depth
